# revision 8
# baseline (speedup 1.0000x reference)
"""RWKV v4 block kernel for 8 TRN2 NeuronCores (nn_Block_15083925144394).

Sharding: data-parallel over batch B=512 -> 64 per core, processed in 2
passes of 32 batch rows. Token-major LN on [100,512] tiles (2 batch rows),
channels-major matmuls/WKV with a 51-wide padded time axis so time-shifts
are plain AP offsets and the WKV recurrence runs as tensor_tensor_scan with
zero-multiplier state resets at batch boundaries.
"""
import os
import sys

sys.path.insert(0, "/opt/trn_rl_repo")

import numpy as np
import ml_dtypes

import concourse.bass as bass
import concourse.mybir as mybir
import concourse.tile as tile
from concourse import bacc
from concourse.bass_utils import run_bass_kernel_spmd
from concourse.masks import make_identity

F32 = mybir.dt.float32
BF16 = mybir.dt.bfloat16
AF = mybir.ActivationFunctionType
OP = mybir.AluOpType

NCORE = 8
B_FULL, T, C, H = 512, 50, 512, 2048
BS = B_FULL // NCORE          # 64 batch rows per core
PB = 16                       # batch rows per pass
NPASS = BS // PB              # 2
TP = T + 1                    # padded time width (col 0 is zero pad)
NT = PB // 2                  # 16 token tiles per pass (2 b-rows x 50 = 100 tokens each)
NTOK = 100                    # tokens per token-tile
CB = C // 128                 # 4 channel blocks
HB = H // 128                 # 16 hidden blocks
BCH = [(0, 10), (10, 16)]     # b-row chunks (<=500 tokens)

_EXEC_NS = [None]


def _build():
    nc = bacc.Bacc("TRN2", target_bir_lowering=False, debug=False, num_devices=NCORE)

    x_d = nc.dram_tensor("x", [BS, T, C], F32, kind="ExternalInput")
    y_d = nc.dram_tensor("y", [BS, T, C], F32, kind="ExternalOutput")
    # weights, lhsT layout [c_in, c_out], bf16
    wd = {}
    for nm, shp in [("wk_a", [C, C]), ("wk_b", [C, C]), ("wv_a", [C, C]),
                    ("wv_b", [C, C]), ("wr_a", [C, C]), ("wr_b", [C, C]),
                    ("wo_t", [C, C]), ("fr_a", [C, C]), ("fr_b", [C, C]),
                    ("fk_t", [C, H]), ("fv_t", [H, C])]:
        wd[nm] = nc.dram_tensor(nm, shp, BF16, kind="ExternalInput")
    colsA_d = nc.dram_tensor("colsA", [128, CB, 4], F32, kind="ExternalInput")   # u, eu, ew, mkf
    colsD_d = nc.dram_tensor("colsD", [128, CB, 8], F32, kind="ExternalInput")   # bk,bkc,bv,bvc,br2,brc2,bfr2,bfrc2
    colsH_d = nc.dram_tensor("colsH", [128, HB, 2], F32, kind="ExternalInput")   # bfk,bfkc

    with tile.TileContext(nc) as tc:
        with tc.tile_pool(name="wpool", bufs=1) as wp, \
             tc.tile_pool(name="big", bufs=1) as bigp, \
             tc.tile_pool(name="med", bufs=1) as medp, \
             tc.tile_pool(name="scr", bufs=2) as scrp, \
             tc.tile_pool(name="st", bufs=2) as stp, \
             tc.tile_pool(name="pmm", bufs=2, space="PSUM") as pmm, \
             tc.tile_pool(name="pkv", bufs=1, space="PSUM") as pkv, \
             tc.tile_pool(name="ptr", bufs=2, space="PSUM") as ptr:

            # ---- persistent constants ----
            ident = wp.tile([128, 128], BF16)
            make_identity(nc, ident[:])
            wt = {}
            for nm in ["wk_a", "wk_b", "wv_a", "wv_b", "wr_a", "wr_b", "wo_t", "fr_a", "fr_b"]:
                wt[nm] = wp.tile([128, CB, C], BF16, tag=nm, name=nm)
                nc.sync.dma_start(wt[nm][:], wd[nm].ap().rearrange("(a p) d -> p a d", p=128))
            wt["fk_t"] = wp.tile([128, CB, H], BF16, tag="fk_t", name="fk_t")
            nc.sync.dma_start(wt["fk_t"][:], wd["fk_t"].ap().rearrange("(a p) d -> p a d", p=128))
            wt["fv_t"] = wp.tile([128, HB, C], BF16, tag="fv_t", name="fv_t")
            nc.sync.dma_start(wt["fv_t"][:], wd["fv_t"].ap().rearrange("(a p) d -> p a d", p=128))
            epsc = wp.tile([128, 1], F32)
            nc.vector.memset(epsc[:], 1e-5)
            colsA = wp.tile([128, CB, 4], F32)
            colsD = wp.tile([128, CB, 8], F32)
            colsH = wp.tile([128, HB, 2], F32)
            nc.sync.dma_start(colsA[:], colsA_d.ap())
            nc.sync.dma_start(colsD[:], colsD_d.ap())
            nc.sync.dma_start(colsH[:], colsH_d.ap())
            u_c = lambda db: colsA[:, db, 0:1]
            eu_c = lambda db: colsA[:, db, 1:2]
            ew_c = lambda db: colsA[:, db, 2:3]

            # ONES feeds the per-db EW rebuild inside the WKV loop
            ONES = wp.tile([128, PB, T], F32)
            nc.vector.memset(ONES[:], 1.0)

            for p in range(NPASS):
                b0 = p * PB
                # ================= Phase A: load + LN1 (token-major) =================
                x_tm = bigp.tile([NTOK, NT, C], F32, tag="xbig")
                for bb in range(PB):
                    nc.sync.dma_start(x_tm[(bb % 2) * T:(bb % 2) * T + T, bb // 2, :],
                                      x_d[b0 + bb])
                S1 = stp.tile([NTOK, NT], F32, tag="s1")
                S2 = stp.tile([NTOK, NT], F32, tag="s2")
                for i in range(NT):
                    sc = scrp.tile([NTOK, C], F32, tag="lnscr")
                    nc.scalar.activation(sc[:], x_tm[:, i, :], AF.Copy,
                                         accum_out=S1[:, i:i + 1])
                    sc2 = scrp.tile([NTOK, C], F32, tag="lnscr")
                    nc.scalar.activation(sc2[:], x_tm[:, i, :], AF.Square,
                                         accum_out=S2[:, i:i + 1])
                MU = stp.tile([NTOK, NT], F32, tag="mu")
                nc.vector.tensor_scalar(MU[:], S1[:], 1.0 / C, None, OP.mult)
                VAR = stp.tile([NTOK, NT], F32, tag="var")
                nc.vector.tensor_mul(VAR[:], MU[:], MU[:])
                nc.vector.scalar_tensor_tensor(VAR[:], S2[:], 1.0 / C, VAR[:],
                                               OP.mult, OP.subtract)
                LV = stp.tile([NTOK, NT], F32, tag="lv")
                nc.scalar.activation(LV[:], VAR[:], AF.Ln, bias=epsc[0:NTOK, :])
                RSTD = stp.tile([NTOK, NT], F32, tag="rstd")
                nc.scalar.activation(RSTD[:], LV[:], AF.Exp, bias=0.0, scale=-0.5)

                h1 = medp.tile([128, CB, PB, TP], BF16, tag="hcm")
                for cb in range(CB):
                    nc.vector.memset(h1[:, cb, :, 0:1], 0.0)
                for i in range(NT):
                    xh = scrp.tile([NTOK, C], F32, tag="xh")
                    nc.vector.tensor_scalar(xh[:], x_tm[:, i, :], MU[:, i:i + 1], None,
                                            OP.subtract)
                    xhb = scrp.tile([NTOK, C], BF16, tag="xhb")
                    nc.vector.tensor_scalar(xhb[:], xh[:], RSTD[:, i:i + 1], None, OP.mult)
                    pst = ptr.tile([128, CB, NTOK], BF16, tag="pst")
                    for cb in range(CB):
                        nc.tensor.transpose(pst[:, cb, :], xhb[:, cb * 128:(cb + 1) * 128],
                                            ident[0:NTOK, 0:NTOK])
                    for cb in range(CB):
                        nc.scalar.copy(h1[:, cb, 2 * i:2 * i + 2, 1:TP],
                                       pst[:, cb, :].rearrange("p (a b) -> p a b", a=2))

                dx = medp.tile([128, CB, PB, T], BF16, tag="dx")
                for cb in range(CB):
                    nc.vector.tensor_sub(dx[:, cb], h1[:, cb, :, 1:TP], h1[:, cb, :, 0:T])

                # ============ Phase B: k/v/r matmuls + WKV, per output block ============
                rwkv = medp.tile([128, CB, PB, TP], BF16, tag="rwkv")
                for db in range(CB):
                    KD = medp.tile([128, PB, TP], F32, tag="kd")
                    VD = medp.tile([128, PB, TP], F32, tag="vd")
                    TH = medp.tile([128, PB, T], BF16, tag="th")
                    for (wa, wb, dst, bcol, ext) in [
                            ("wk_a", "wk_b", KD, 0, True),
                            ("wv_a", "wv_b", VD, 2, True),
                            ("wr_a", "wr_b", TH, 4, False)]:
                        for (bl, bh) in BCH:
                            nb = bh - bl
                            ps = pmm.tile([128, 10, T], F32, tag="ps")
                            pso = ps[:, 0:nb, :].rearrange("p a b -> p (a b)")
                            for ci in range(CB):
                                nc.tensor.matmul(pso, wt[wa][:, ci, db * 128:(db + 1) * 128],
                                                 h1[:, ci, bl:bh, 0:T],
                                                 start=(ci == 0), stop=False)
                            for ci in range(CB):
                                nc.tensor.matmul(pso, wt[wb][:, ci, db * 128:(db + 1) * 128],
                                                 dx[:, ci, bl:bh, :],
                                                 start=False, stop=(ci == CB - 1))
                            if ext:  # k/v: affine evac with t=0 bias correction
                                nc.scalar.activation(dst[:, bl:bh, 2:TP], ps[:, 0:nb, 1:T],
                                                     AF.Identity, bias=colsD[:, db, bcol:bcol + 1])
                                nc.scalar.activation(dst[:, bl:bh, 1:2], ps[:, 0:nb, 0:1],
                                                     AF.Identity, bias=colsD[:, db, bcol + 1:bcol + 2])
                            else:  # r: tanh(0.5 r + 0.5 bias) directly
                                nc.scalar.activation(dst[:, bl:bh, 1:T], ps[:, 0:nb, 1:T],
                                                     AF.Tanh, bias=colsD[:, db, 4:5], scale=0.5)
                                nc.scalar.activation(dst[:, bl:bh, 0:1], ps[:, 0:nb, 0:1],
                                                     AF.Tanh, bias=colsD[:, db, 5:6], scale=0.5)
                    # WKV chain for this block
                    EK = medp.tile([128, PB, TP], F32, tag="ek")
                    EUK = medp.tile([128, PB, TP], F32, tag="euk")
                    nc.scalar.activation(EK[:, :, 1:TP], KD[:, :, 1:TP], AF.Exp)
                    nc.scalar.activation(EUK[:, :, 1:TP], KD[:, :, 1:TP], AF.Exp, bias=u_c(db))
                    EKV = medp.tile([128, PB, TP], F32, tag="ekv")
                    nc.vector.tensor_mul(EKV[:, :, 1:TP], EK[:, :, 1:TP], VD[:, :, 1:TP])
                    EUKV = medp.tile([128, PB, TP], F32, tag="eukv")
                    nc.vector.tensor_scalar(EUKV[:, :, 1:TP], EKV[:, :, 1:TP], eu_c(db),
                                            None, OP.mult)
                    nc.vector.memset(EK[:, :, 0:1], 0.0)
                    nc.vector.memset(EKV[:, :, 0:1], 0.0)
                    EWd = medp.tile([128, PB, TP], F32, tag="ewd")
                    nc.vector.tensor_scalar(EWd[:, :, 1:TP], ONES[:], ew_c(db), None, OP.mult)
                    nc.vector.memset(EWd[:, :, 0:1], 0.0)
                    A = medp.tile([128, PB, TP], F32, tag="a")
                    nc.vector.tensor_tensor_scan(A.rearrange("p b t -> p (b t)"),
                                                 EWd.rearrange("p b t -> p (b t)"),
                                                 EKV.rearrange("p b t -> p (b t)"),
                                                 0.0, OP.mult, OP.add)
                    BB = medp.tile([128, PB, TP], F32, tag="bb")
                    nc.vector.tensor_tensor_scan(BB.rearrange("p b t -> p (b t)"),
                                                 EWd.rearrange("p b t -> p (b t)"),
                                                 EK.rearrange("p b t -> p (b t)"),
                                                 0.0, OP.mult, OP.add)
                    NUM = medp.tile([128, PB, T], F32, tag="num")
                    nc.vector.tensor_add(NUM[:], A[:, :, 0:T], EUKV[:, :, 1:TP])
                    DEN = medp.tile([128, PB, T], F32, tag="den")
                    nc.vector.tensor_add(DEN[:], BB[:, :, 0:T], EUK[:, :, 1:TP])
                    LD = medp.tile([128, PB, T], F32, tag="ld")
                    nc.scalar.activation(LD[:], DEN[:], AF.Ln)
                    RCP = medp.tile([128, PB, T], F32, tag="rcp")
                    nc.scalar.activation(RCP[:], LD[:], AF.Exp, bias=0.0, scale=-1.0)
                    Y = medp.tile([128, PB, T], F32, tag="y")
                    nc.vector.tensor_mul(Y[:], NUM[:], RCP[:])
                    nc.vector.scalar_tensor_tensor(rwkv[:, db, :, 1:TP], TH[:], 1.0, Y[:],
                                                   OP.add, OP.mult)

                # ============ att = Wo @ rwkv, transpose back, residual ============
                attc = medp.tile([128, CB, PB, T], BF16, tag="dx")
                for db in range(CB):
                    for (bl, bh) in BCH:
                        nb = bh - bl
                        ps = pmm.tile([128, 10, T], F32, tag="ps")
                        pso = ps[:, 0:nb, :].rearrange("p a b -> p (a b)")
                        for ci in range(CB):
                            nc.tensor.matmul(pso, wt["wo_t"][:, ci, db * 128:(db + 1) * 128],
                                             rwkv[:, ci, bl:bh, 1:TP],
                                             start=(ci == 0), stop=(ci == CB - 1))
                        nc.scalar.copy(attc[:, db, bl:bh, :].rearrange("p a b -> p (a b)"),
                                       ps[:, 0:nb, :].rearrange("p a b -> p (a b)"))
                out1 = bigp.tile([NTOK, NT, C], F32, tag="out1")
                for i in range(NT):
                    psb = ptr.tile([NTOK, CB, 128], BF16, tag="pst")
                    for cb in range(CB):
                        nc.tensor.transpose(psb[:, cb, :],
                                            attc[:, cb, 2 * i:2 * i + 2, :]
                                            .rearrange("p a b -> p (a b)"),
                                            ident[:])
                    nc.vector.scalar_tensor_tensor(out1[:, i, :],
                                                   psb.rearrange("p a b -> p (a b)"),
                                                   1.0, x_tm[:, i, :], OP.mult, OP.add)

                # ================= Phase C: LN2 (token-major) =================
                for i in range(NT):
                    sc = scrp.tile([NTOK, C], F32, tag="lnscr")
                    nc.scalar.activation(sc[:], out1[:, i, :], AF.Copy,
                                         accum_out=S1[:, i:i + 1])
                    sc2 = scrp.tile([NTOK, C], F32, tag="lnscr")
                    nc.scalar.activation(sc2[:], out1[:, i, :], AF.Square,
                                         accum_out=S2[:, i:i + 1])
                MU2 = stp.tile([NTOK, NT], F32, tag="mu")
                nc.vector.tensor_scalar(MU2[:], S1[:], 1.0 / C, None, OP.mult)
                VAR2 = stp.tile([NTOK, NT], F32, tag="var")
                nc.vector.tensor_mul(VAR2[:], MU2[:], MU2[:])
                nc.vector.scalar_tensor_tensor(VAR2[:], S2[:], 1.0 / C, VAR2[:],
                                               OP.mult, OP.subtract)
                LV2 = stp.tile([NTOK, NT], F32, tag="lv")
                nc.scalar.activation(LV2[:], VAR2[:], AF.Ln, bias=epsc[0:NTOK, :])
                RSTD2 = stp.tile([NTOK, NT], F32, tag="rstd")
                nc.scalar.activation(RSTD2[:], LV2[:], AF.Exp, bias=0.0, scale=-0.5)
                h2 = medp.tile([128, CB, PB, TP], BF16, tag="hcm2")
                for cb in range(CB):
                    nc.vector.memset(h2[:, cb, :, 0:1], 0.0)
                for i in range(NT):
                    xh = scrp.tile([NTOK, C], F32, tag="xh")
                    nc.vector.tensor_scalar(xh[:], out1[:, i, :], MU2[:, i:i + 1], None,
                                            OP.subtract)
                    xhb = scrp.tile([NTOK, C], BF16, tag="xhb")
                    nc.vector.tensor_scalar(xhb[:], xh[:], RSTD2[:, i:i + 1], None, OP.mult)
                    pst = ptr.tile([128, CB, NTOK], BF16, tag="pst")
                    for cb in range(CB):
                        nc.tensor.transpose(pst[:, cb, :], xhb[:, cb * 128:(cb + 1) * 128],
                                            ident[0:NTOK, 0:NTOK])
                    for cb in range(CB):
                        nc.scalar.copy(h2[:, cb, 2 * i:2 * i + 2, 1:TP],
                                       pst[:, cb, :].rearrange("p (a b) -> p a b", a=2))
                dx2 = medp.tile([128, CB, PB, T], BF16, tag="dx")
                for cb in range(CB):
                    nc.vector.tensor_sub(dx2[:, cb], h2[:, cb, :, 1:TP], h2[:, cb, :, 0:T])

                # ============ Phase D: FFN ============
                # fr path: frr = Fr@(h2sh + mrf*dx2) -> th2 = tanh(0.5 frr + 0.5 bias)
                th2 = medp.tile([128, CB, PB, T], BF16, tag="th2")
                for db in range(CB):
                    for (bl, bh) in BCH:
                        nb = bh - bl
                        ps = pmm.tile([128, 10, T], F32, tag="ps")
                        pso = ps[:, 0:nb, :].rearrange("p a b -> p (a b)")
                        for ci in range(CB):
                            nc.tensor.matmul(pso, wt["fr_a"][:, ci, db * 128:(db + 1) * 128],
                                             h2[:, ci, bl:bh, 0:T],
                                             start=(ci == 0), stop=False)
                        for ci in range(CB):
                            nc.tensor.matmul(pso, wt["fr_b"][:, ci, db * 128:(db + 1) * 128],
                                             dx2[:, ci, bl:bh, :],
                                             start=False, stop=(ci == CB - 1))
                        nc.scalar.activation(th2[:, db, bl:bh, 1:T], ps[:, 0:nb, 1:T],
                                             AF.Tanh, bias=colsD[:, db, 6:7], scale=0.5)
                        nc.scalar.activation(th2[:, db, bl:bh, 0:1], ps[:, 0:nb, 0:1],
                                             AF.Tanh, bias=colsD[:, db, 7:8], scale=0.5)
                # fk / fv path with relu^2, streamed per h-block
                fkm = medp.tile([128, CB, PB, TP], BF16, tag="rwkv")
                for ci in range(CB):
                    nc.vector.scalar_tensor_tensor(fkm[:, ci, :, 1:TP], dx2[:, ci],
                                                   colsA[:, ci, 3:4], h2[:, ci, :, 0:T],
                                                   OP.mult, OP.add)
                rkv = medp.tile([128, CB, PB, T], BF16, tag="rkv")
                for (bl, bh) in BCH:
                    nb = bh - bl
                    pvs = [pkv.tile([128, 10, T], F32, tag=f"kv{cb}", name=f"kv{cb}") for cb in range(CB)]
                    for hb in range(HB):
                        ps = pmm.tile([128, 10, T], F32, tag="ps")
                        pso = ps[:, 0:nb, :].rearrange("p a b -> p (a b)")
                        for ci in range(CB):
                            nc.tensor.matmul(pso, wt["fk_t"][:, ci, hb * 128:(hb + 1) * 128],
                                             fkm[:, ci, bl:bh, 1:TP],
                                             start=(ci == 0), stop=(ci == CB - 1))
                        tkk = scrp.tile([128, 10, T], F32, tag="tkk")
                        nc.scalar.activation(tkk[:, 0:nb, 1:T], ps[:, 0:nb, 1:T],
                                             AF.Relu, bias=colsH[:, hb, 0:1])
                        nc.scalar.activation(tkk[:, 0:nb, 0:1], ps[:, 0:nb, 0:1],
                                             AF.Relu, bias=colsH[:, hb, 1:2])
                        kk = scrp.tile([128, 10, T], BF16, tag="kk")
                        nc.vector.tensor_mul(kk[:, 0:nb, :], tkk[:, 0:nb, :], tkk[:, 0:nb, :])
                        for cb in range(CB):
                            nc.tensor.matmul(pvs[cb][:, 0:nb, :].rearrange("p a b -> p (a b)"),
                                             wt["fv_t"][:, hb, cb * 128:(cb + 1) * 128],
                                             kk[:, 0:nb, :].rearrange("p a b -> p (a b)"),
                                             start=(hb == 0), stop=(hb == HB - 1))
                    for cb in range(CB):
                        nc.vector.scalar_tensor_tensor(rkv[:, cb, bl:bh, :], th2[:, cb, bl:bh, :],
                                                       1.0, pvs[cb][:, 0:nb, :],
                                                       OP.add, OP.mult)

                # ============ final: out2 = out1 + rkv^T, DMA out ============
                out2 = bigp.tile([NTOK, NT, C], F32, tag="xbig")
                for i in range(NT):
                    psb = ptr.tile([NTOK, CB, 128], BF16, tag="pst")
                    for cb in range(CB):
                        nc.tensor.transpose(psb[:, cb, :],
                                            rkv[:, cb, 2 * i:2 * i + 2, :]
                                            .rearrange("p a b -> p (a b)"),
                                            ident[:])
                    nc.vector.scalar_tensor_tensor(out2[:, i, :],
                                                   psb.rearrange("p a b -> p (a b)"),
                                                   1.0, out1[:, i, :], OP.mult, OP.add)
                for bb in range(PB):
                    nc.sync.dma_start(y_d[b0 + bb],
                                      out2[(bb % 2) * T:(bb % 2) * T + T, bb // 2, :])

    nc.compile()
    return nc


def _prep_inputs(inputs):
    bf = ml_dtypes.bfloat16
    f64 = np.float64
    g1 = np.asarray(inputs["ln1_g"], f64)
    b1 = np.asarray(inputs["ln1_b"], f64)
    g2 = np.asarray(inputs["ln2_g"], f64)
    b2 = np.asarray(inputs["ln2_b"], f64)
    mk = np.asarray(inputs["att_mix_k"], f64).ravel()
    mv = np.asarray(inputs["att_mix_v"], f64).ravel()
    mr = np.asarray(inputs["att_mix_r"], f64).ravel()
    mkf = np.asarray(inputs["ffn_mix_k"], f64).ravel()
    mrf = np.asarray(inputs["ffn_mix_r"], f64).ravel()
    td = np.asarray(inputs["time_decay"], f64)
    u = np.asarray(inputs["time_first"], f64)
    Wk = np.asarray(inputs["Wk"], f64)
    Wv = np.asarray(inputs["Wv"], f64)
    Wr = np.asarray(inputs["Wr"], f64)
    Wo = np.asarray(inputs["Wo"], f64)
    Fk = np.asarray(inputs["Fk"], f64)
    Fv = np.asarray(inputs["Fv"], f64)
    Fr = np.asarray(inputs["Fr"], f64)

    def lhsT(W, colscale):
        return np.ascontiguousarray((W * colscale[None, :]).T.astype(np.float32)).astype(bf)

    d = {}
    d["wk_a"] = lhsT(Wk, g1)
    d["wk_b"] = lhsT(Wk, g1 * mk)
    d["wv_a"] = lhsT(Wv, g1)
    d["wv_b"] = lhsT(Wv, g1 * mv)
    d["wr_a"] = lhsT(Wr, g1)
    d["wr_b"] = lhsT(Wr, g1 * mr)
    d["wo_t"] = lhsT(0.5 * Wo, np.ones(C))
    d["fr_a"] = lhsT(Fr, g2)
    d["fr_b"] = lhsT(Fr, g2 * mrf)
    d["fk_t"] = lhsT(Fk, g2)
    d["fv_t"] = lhsT(0.5 * Fv, np.ones(H))

    def cols(vecs):
        # [C or H] vectors -> [128, nblk, nvec]
        n = vecs[0].shape[0]
        arr = np.stack(vecs, -1).reshape(n // 128, 128, len(vecs))
        return np.ascontiguousarray(arr.transpose(1, 0, 2)).astype(np.float32)

    ew = np.exp(-np.exp(td))
    eu = np.exp(u)
    d["colsA"] = cols([u, eu, ew, mkf])
    bk = Wk @ b1
    bkc = Wk @ (mk * b1)
    bv = Wv @ b1
    bvc = Wv @ (mv * b1)
    br = Wr @ b1
    brc = Wr @ (mr * b1)
    bfr = Fr @ b2
    bfrc = Fr @ (mrf * b2)
    d["colsD"] = cols([bk, bkc, bv, bvc, 0.5 * br, 0.5 * brc, 0.5 * bfr, 0.5 * bfrc])
    bfk = Fk @ b2
    bfkc = Fk @ (mkf * b2)
    d["colsH"] = cols([bfk, bfkc])
    return d


_NC_CACHE = [None]


def kernel(**inputs):
    if _NC_CACHE[0] is None:
        _NC_CACHE[0] = _build()
    nc = _NC_CACHE[0]
    d = _prep_inputs(inputs)
    x = np.asarray(inputs["x"], np.float32)
    in_maps = []
    for c in range(NCORE):
        m = dict(d)
        m["x"] = np.ascontiguousarray(x[c * BS:(c + 1) * BS])
        in_maps.append(m)
    trace = os.environ.get("RWKV_TRACE") == "1"
    res = run_bass_kernel_spmd(nc, in_maps, list(range(NCORE)), trace=trace)
    if trace:
        _EXEC_NS[0] = res.exec_time_ns
    return np.concatenate([r["y"] for r in res.results], axis=0)


# revision 12
# speedup vs baseline: 1.1599x; 1.1599x over previous
"""RWKV v4 block kernel for 8 TRN2 NeuronCores (nn_Block_15083925144394).

Sharding: data-parallel over batch B=512 -> 64 per core, processed in 2
passes of 32 batch rows. Token-major LN on [100,512] tiles (2 batch rows),
channels-major matmuls/WKV with a 51-wide padded time axis so time-shifts
are plain AP offsets and the WKV recurrence runs as tensor_tensor_scan with
zero-multiplier state resets at batch boundaries.
"""
import os
import sys

sys.path.insert(0, "/opt/trn_rl_repo")

import numpy as np
import ml_dtypes

import concourse.bass as bass
import concourse.mybir as mybir
import concourse.tile as tile
from concourse import bacc
from concourse.bass_utils import run_bass_kernel_spmd
from concourse.masks import make_identity

F32 = mybir.dt.float32
BF16 = mybir.dt.bfloat16
AF = mybir.ActivationFunctionType
OP = mybir.AluOpType

NCORE = 8
B_FULL, T, C, H = 512, 50, 512, 2048
BS = B_FULL // NCORE          # 64 batch rows per core
PB = 16                       # batch rows per pass
NPASS = BS // PB              # 2
TP = T + 1                    # padded time width (col 0 is zero pad)
NT = PB // 2                  # 16 token tiles per pass (2 b-rows x 50 = 100 tokens each)
NTOK = 100                    # tokens per token-tile
CB = C // 128                 # 4 channel blocks
HB = H // 128                 # 16 hidden blocks
BCH = [(0, 10), (10, 16)]     # b-row chunks (<=500 tokens)

_EXEC_NS = [None]


def _build():
    nc = bacc.Bacc("TRN2", target_bir_lowering=False, debug=False, num_devices=NCORE)

    x_d = nc.dram_tensor("x", [BS, T, C], F32, kind="ExternalInput")
    y_d = nc.dram_tensor("y", [BS, T, C], F32, kind="ExternalOutput")
    # weights, lhsT layout [c_in, c_out], bf16
    wd = {}
    for nm, shp in [("wk_a", [C, C]), ("wk_b", [C, C]), ("wv_a", [C, C]),
                    ("wv_b", [C, C]), ("wr_a", [C, C]), ("wr_b", [C, C]),
                    ("wo_t", [C, C]), ("fr_a", [C, C]), ("fr_b", [C, C]),
                    ("fk_t", [C, H]), ("fv_t", [H, C])]:
        wd[nm] = nc.dram_tensor(nm, shp, BF16, kind="ExternalInput")
    colsA_d = nc.dram_tensor("colsA", [128, CB, 5], F32, kind="ExternalInput")   # u, eu, ew, mkf, 1-mkf
    colsD_d = nc.dram_tensor("colsD", [128, CB, 8], F32, kind="ExternalInput")   # bk,bkc,bv,bvc,br2,brc2,bfr2,bfrc2
    colsH_d = nc.dram_tensor("colsH", [128, HB, 2], F32, kind="ExternalInput")   # bfk,bfkc

    with tile.TileContext(nc) as tc:
        with tc.tile_pool(name="wpool", bufs=1) as wp, \
             tc.tile_pool(name="big", bufs=1) as bigp, \
             tc.tile_pool(name="med", bufs=1) as medp, \
             tc.tile_pool(name="scr", bufs=2) as scrp, \
             tc.tile_pool(name="st", bufs=2) as stp, \
             tc.tile_pool(name="pmm", bufs=2, space="PSUM") as pmm, \
             tc.tile_pool(name="pkv", bufs=1, space="PSUM") as pkv, \
             tc.tile_pool(name="ptr", bufs=2, space="PSUM") as ptr:

            # ---- persistent constants ----
            ident = wp.tile([128, 128], BF16)
            make_identity(nc, ident[:])
            wt = {}
            for nm in ["wk_a", "wk_b", "wv_a", "wv_b", "wr_a", "wr_b", "wo_t", "fr_a", "fr_b"]:
                wt[nm] = wp.tile([128, CB, C], BF16, tag=nm, name=nm)
                nc.sync.dma_start(wt[nm][:], wd[nm].ap().rearrange("(a p) d -> p a d", p=128))
            wt["fk_t"] = wp.tile([128, CB, H], BF16, tag="fk_t", name="fk_t")
            nc.sync.dma_start(wt["fk_t"][:], wd["fk_t"].ap().rearrange("(a p) d -> p a d", p=128))
            wt["fv_t"] = wp.tile([128, HB, C], BF16, tag="fv_t", name="fv_t")
            nc.sync.dma_start(wt["fv_t"][:], wd["fv_t"].ap().rearrange("(a p) d -> p a d", p=128))
            epsc = wp.tile([128, 1], F32)
            nc.vector.memset(epsc[:], 1e-5)
            colsA = wp.tile([128, CB, 5], F32)
            colsD = wp.tile([128, CB, 8], F32)
            colsH = wp.tile([128, HB, 2], F32)
            nc.sync.dma_start(colsA[:], colsA_d.ap())
            nc.sync.dma_start(colsD[:], colsD_d.ap())
            nc.sync.dma_start(colsH[:], colsH_d.ap())
            u_c = lambda db: colsA[:, db, 0:1]
            eu_c = lambda db: colsA[:, db, 1:2]
            ew_c = lambda db: colsA[:, db, 2:3]

            # ONES feeds the per-db EW rebuild inside the WKV loop
            ONES = wp.tile([128, PB, T], F32)
            nc.vector.memset(ONES[:], 1.0)

            for p in range(NPASS):
                b0 = p * PB
                # ================= Phase A: load + LN1 (token-major) =================
                x_tm = bigp.tile([NTOK, NT, C], F32, tag="xbig")
                for bb in range(PB):
                    nc.sync.dma_start(x_tm[(bb % 2) * T:(bb % 2) * T + T, bb // 2, :],
                                      x_d[b0 + bb])
                MV = stp.tile([NTOK, NT, 2], F32, tag="mv")
                for i in range(NT):
                    bst = stp.tile([NTOK, 6], F32, tag="bst")
                    nc.vector.bn_stats(bst[:], x_tm[:, i, :])
                    nc.vector.bn_aggr(MV[:, i, :], bst[:])
                LV = stp.tile([NTOK, NT], F32, tag="lv")
                nc.scalar.activation(LV[:], MV[:, :, 1:2], AF.Ln, bias=epsc[0:NTOK, :])
                RSTD = stp.tile([NTOK, NT], F32, tag="rstd")
                nc.scalar.activation(RSTD[:], LV[:], AF.Exp, bias=0.0, scale=-0.5)

                h1 = medp.tile([128, CB, PB, TP], BF16, tag="hcm")
                for cb in range(CB):
                    nc.vector.memset(h1[:, cb, :, 0:1], 0.0)
                for i in range(NT):
                    xh = scrp.tile([NTOK, C], F32, tag="xh")
                    nc.vector.tensor_scalar(xh[:], x_tm[:, i, :], MV[:, i, 0:1], None,
                                            OP.subtract)
                    xhb = scrp.tile([NTOK, C], BF16, tag="xhb")
                    nc.vector.tensor_scalar(xhb[:], xh[:], RSTD[:, i:i + 1], None, OP.mult)
                    pst = ptr.tile([128, CB, NTOK], BF16, tag="pst")
                    for cb in range(CB):
                        nc.tensor.transpose(pst[:, cb, :], xhb[:, cb * 128:(cb + 1) * 128],
                                            ident[0:NTOK, 0:NTOK])
                    for cb in range(CB):
                        nc.scalar.copy(h1[:, cb, 2 * i:2 * i + 2, 1:TP],
                                       pst[:, cb, :].rearrange("p (a b) -> p a b", a=2))


                # ============ Phase B: k/v/r matmuls + WKV, per output block ============
                rwkv = medp.tile([128, CB, PB, TP], BF16, tag="rwkv")
                for db in range(CB):
                    KD = medp.tile([128, PB, TP], F32, tag="kd")
                    VD = medp.tile([128, PB, TP], F32, tag="vd")
                    TH = medp.tile([128, PB, T], F32, tag="th")
                    for (wa, wb, dst, bcol, ext) in [
                            ("wk_a", "wk_b", KD, 0, True),
                            ("wv_a", "wv_b", VD, 2, True),
                            ("wr_a", "wr_b", TH, 4, False)]:
                        for (bl, bh) in BCH:
                            nb = bh - bl
                            ps = pmm.tile([128, 10, T], F32, tag="ps")
                            pso = ps[:, 0:nb, :].rearrange("p a b -> p (a b)")
                            for ci in range(CB):
                                nc.tensor.matmul(pso, wt[wa][:, ci, db * 128:(db + 1) * 128],
                                                 h1[:, ci, bl:bh, 0:T],
                                                 start=(ci == 0), stop=False)
                            for ci in range(CB):
                                nc.tensor.matmul(pso, wt[wb][:, ci, db * 128:(db + 1) * 128],
                                                 h1[:, ci, bl:bh, 1:TP],
                                                 start=False, stop=(ci == CB - 1))
                            if ext:  # k/v: affine evac with t=0 bias correction
                                nc.scalar.activation(dst[:, bl:bh, 2:TP], ps[:, 0:nb, 1:T],
                                                     AF.Identity, bias=colsD[:, db, bcol:bcol + 1])
                                nc.scalar.activation(dst[:, bl:bh, 1:2], ps[:, 0:nb, 0:1],
                                                     AF.Identity, bias=colsD[:, db, bcol + 1:bcol + 2])
                            else:  # r: E3 = exp(-(r + bias)) for sigmoid-fold
                                nc.scalar.activation(dst[:, bl:bh, 1:T], ps[:, 0:nb, 1:T],
                                                     AF.Exp, bias=colsD[:, db, 4:5], scale=-1.0)
                                nc.scalar.activation(dst[:, bl:bh, 0:1], ps[:, 0:nb, 0:1],
                                                     AF.Exp, bias=colsD[:, db, 5:6], scale=-1.0)
                    # WKV chain for this block
                    EK = medp.tile([128, PB, TP], F32, tag="ek")
                    EUK = medp.tile([128, PB, TP], F32, tag="euk")
                    nc.scalar.activation(EK[:, :, 1:TP], KD[:, :, 1:TP], AF.Exp)
                    nc.vector.tensor_scalar(EUK[:, :, 1:TP], EK[:, :, 1:TP], eu_c(db),
                                            None, OP.mult)
                    EKV = medp.tile([128, PB, TP], F32, tag="ekv")
                    nc.vector.tensor_mul(EKV[:, :, 1:TP], EK[:, :, 1:TP], VD[:, :, 1:TP])
                    EUKV = medp.tile([128, PB, TP], F32, tag="eukv")
                    nc.vector.tensor_scalar(EUKV[:, :, 1:TP], EKV[:, :, 1:TP], eu_c(db),
                                            None, OP.mult)
                    nc.vector.memset(EK[:, :, 0:1], 0.0)
                    nc.vector.memset(EKV[:, :, 0:1], 0.0)
                    EWd = medp.tile([128, PB, TP], F32, tag="ewd")
                    nc.vector.tensor_scalar(EWd[:, :, 1:TP], ONES[:], ew_c(db), None, OP.mult)
                    nc.vector.memset(EWd[:, :, 0:1], 0.0)
                    A = medp.tile([128, PB, TP], F32, tag="a")
                    nc.vector.tensor_tensor_scan(A.rearrange("p b t -> p (b t)"),
                                                 EWd.rearrange("p b t -> p (b t)"),
                                                 EKV.rearrange("p b t -> p (b t)"),
                                                 0.0, OP.mult, OP.add)
                    BB = medp.tile([128, PB, TP], F32, tag="bb")
                    nc.vector.tensor_tensor_scan(BB.rearrange("p b t -> p (b t)"),
                                                 EWd.rearrange("p b t -> p (b t)"),
                                                 EK.rearrange("p b t -> p (b t)"),
                                                 0.0, OP.mult, OP.add)
                    NUM = medp.tile([128, PB, T], F32, tag="num")
                    nc.vector.tensor_add(NUM[:], A[:, :, 0:T], EUKV[:, :, 1:TP])
                    DEN = medp.tile([128, PB, T], F32, tag="den")
                    nc.vector.tensor_add(DEN[:], BB[:, :, 0:T], EUK[:, :, 1:TP])
                    LD = medp.tile([128, PB, T], F32, tag="ld")
                    nc.scalar.activation(LD[:], DEN[:], AF.Ln)
                    L2 = medp.tile([128, PB, T], F32, tag="y")
                    nc.scalar.activation(L2[:], TH[:], AF.Ln, bias=1.0)
                    LSUM = medp.tile([128, PB, T], F32, tag="rcp")
                    nc.vector.tensor_add(LSUM[:], LD[:], L2[:])
                    RCP = medp.tile([128, PB, T], F32, tag="ld2")
                    nc.scalar.activation(RCP[:], LSUM[:], AF.Exp, bias=0.0, scale=-1.0)
                    nc.vector.tensor_mul(rwkv[:, db, :, 1:TP], NUM[:], RCP[:])

                # ============ att = Wo @ rwkv, transpose back, residual ============
                attc = medp.tile([128, CB, PB, T], BF16, tag="dx")
                for db in range(CB):
                    for (bl, bh) in BCH:
                        nb = bh - bl
                        ps = pmm.tile([128, 10, T], F32, tag="ps")
                        pso = ps[:, 0:nb, :].rearrange("p a b -> p (a b)")
                        for ci in range(CB):
                            nc.tensor.matmul(pso, wt["wo_t"][:, ci, db * 128:(db + 1) * 128],
                                             rwkv[:, ci, bl:bh, 1:TP],
                                             start=(ci == 0), stop=(ci == CB - 1))
                        nc.scalar.copy(attc[:, db, bl:bh, :].rearrange("p a b -> p (a b)"),
                                       ps[:, 0:nb, :].rearrange("p a b -> p (a b)"))
                out1 = bigp.tile([NTOK, NT, C], F32, tag="out1")
                for i in range(NT):
                    psb = ptr.tile([NTOK, CB, 128], BF16, tag="pst")
                    for cb in range(CB):
                        nc.tensor.transpose(psb[:, cb, :],
                                            attc[:, cb, 2 * i:2 * i + 2, :]
                                            .rearrange("p a b -> p (a b)"),
                                            ident[:])
                    nc.vector.scalar_tensor_tensor(out1[:, i, :],
                                                   psb.rearrange("p a b -> p (a b)"),
                                                   1.0, x_tm[:, i, :], OP.mult, OP.add)

                # ================= Phase C: LN2 (token-major) =================
                MV2 = stp.tile([NTOK, NT, 2], F32, tag="mv")
                for i in range(NT):
                    bst = stp.tile([NTOK, 6], F32, tag="bst")
                    nc.vector.bn_stats(bst[:], out1[:, i, :])
                    nc.vector.bn_aggr(MV2[:, i, :], bst[:])
                LV2 = stp.tile([NTOK, NT], F32, tag="lv")
                nc.scalar.activation(LV2[:], MV2[:, :, 1:2], AF.Ln, bias=epsc[0:NTOK, :])
                RSTD2 = stp.tile([NTOK, NT], F32, tag="rstd")
                nc.scalar.activation(RSTD2[:], LV2[:], AF.Exp, bias=0.0, scale=-0.5)
                h2 = medp.tile([128, CB, PB, TP], BF16, tag="hcm2")
                for cb in range(CB):
                    nc.vector.memset(h2[:, cb, :, 0:1], 0.0)
                for i in range(NT):
                    xh = scrp.tile([NTOK, C], F32, tag="xh")
                    nc.vector.tensor_scalar(xh[:], out1[:, i, :], MV2[:, i, 0:1], None,
                                            OP.subtract)
                    xhb = scrp.tile([NTOK, C], BF16, tag="xhb")
                    nc.vector.tensor_scalar(xhb[:], xh[:], RSTD2[:, i:i + 1], None, OP.mult)
                    pst = ptr.tile([128, CB, NTOK], BF16, tag="pst")
                    for cb in range(CB):
                        nc.tensor.transpose(pst[:, cb, :], xhb[:, cb * 128:(cb + 1) * 128],
                                            ident[0:NTOK, 0:NTOK])
                    for cb in range(CB):
                        nc.scalar.copy(h2[:, cb, 2 * i:2 * i + 2, 1:TP],
                                       pst[:, cb, :].rearrange("p (a b) -> p a b", a=2))

                # ============ Phase D: FFN ============
                # fr path: frr = Fr@(h2sh + mrf*dx2) -> th2 = tanh(0.5 frr + 0.5 bias)
                th2 = medp.tile([128, CB, PB, T], BF16, tag="th2")
                for db in range(CB):
                    for (bl, bh) in BCH:
                        nb = bh - bl
                        ps = pmm.tile([128, 10, T], F32, tag="ps")
                        pso = ps[:, 0:nb, :].rearrange("p a b -> p (a b)")
                        for ci in range(CB):
                            nc.tensor.matmul(pso, wt["fr_a"][:, ci, db * 128:(db + 1) * 128],
                                             h2[:, ci, bl:bh, 0:T],
                                             start=(ci == 0), stop=False)
                        for ci in range(CB):
                            nc.tensor.matmul(pso, wt["fr_b"][:, ci, db * 128:(db + 1) * 128],
                                             h2[:, ci, bl:bh, 1:TP],
                                             start=False, stop=(ci == CB - 1))
                        nc.scalar.activation(th2[:, db, bl:bh, 1:T], ps[:, 0:nb, 1:T],
                                             AF.Tanh, bias=colsD[:, db, 6:7], scale=0.5)
                        nc.scalar.activation(th2[:, db, bl:bh, 0:1], ps[:, 0:nb, 0:1],
                                             AF.Tanh, bias=colsD[:, db, 7:8], scale=0.5)
                # fk / fv path with relu^2, streamed per h-block
                fkm = medp.tile([128, CB, PB, TP], BF16, tag="rwkv")
                for ci in range(CB):
                    fct = scrp.tile([128, PB, T], BF16, tag="fct")
                    nc.vector.tensor_scalar(fct[:], h2[:, ci, :, 1:TP], colsA[:, ci, 3:4],
                                            None, OP.mult)
                    nc.vector.scalar_tensor_tensor(fkm[:, ci, :, 1:TP], h2[:, ci, :, 0:T],
                                                   colsA[:, ci, 4:5], fct[:],
                                                   OP.mult, OP.add)
                rkv = medp.tile([128, CB, PB, T], BF16, tag="rkv")
                for (bl, bh) in BCH:
                    nb = bh - bl
                    pvs = [pkv.tile([128, 10, T], F32, tag=f"kv{cb}", name=f"kv{cb}") for cb in range(CB)]
                    for hb in range(HB):
                        ps = pmm.tile([128, 10, T], F32, tag="ps")
                        pso = ps[:, 0:nb, :].rearrange("p a b -> p (a b)")
                        for ci in range(CB):
                            nc.tensor.matmul(pso, wt["fk_t"][:, ci, hb * 128:(hb + 1) * 128],
                                             fkm[:, ci, bl:bh, 1:TP],
                                             start=(ci == 0), stop=(ci == CB - 1))
                        tkk = scrp.tile([128, 10, T], F32, tag="tkk")
                        nc.scalar.activation(tkk[:, 0:nb, 1:T], ps[:, 0:nb, 1:T],
                                             AF.Relu, bias=colsH[:, hb, 0:1])
                        nc.scalar.activation(tkk[:, 0:nb, 0:1], ps[:, 0:nb, 0:1],
                                             AF.Relu, bias=colsH[:, hb, 1:2])
                        kk = scrp.tile([128, 10, T], BF16, tag="kk")
                        nc.vector.tensor_mul(kk[:, 0:nb, :], tkk[:, 0:nb, :], tkk[:, 0:nb, :])
                        for cb in range(CB):
                            nc.tensor.matmul(pvs[cb][:, 0:nb, :].rearrange("p a b -> p (a b)"),
                                             wt["fv_t"][:, hb, cb * 128:(cb + 1) * 128],
                                             kk[:, 0:nb, :].rearrange("p a b -> p (a b)"),
                                             start=(hb == 0), stop=(hb == HB - 1))
                    for cb in range(CB):
                        nc.vector.scalar_tensor_tensor(rkv[:, cb, bl:bh, :], th2[:, cb, bl:bh, :],
                                                       1.0, pvs[cb][:, 0:nb, :],
                                                       OP.add, OP.mult)

                # ============ final: out2 = out1 + rkv^T, DMA out ============
                out2 = bigp.tile([NTOK, NT, C], F32, tag="xbig")
                for i in range(NT):
                    psb = ptr.tile([NTOK, CB, 128], BF16, tag="pst")
                    for cb in range(CB):
                        nc.tensor.transpose(psb[:, cb, :],
                                            rkv[:, cb, 2 * i:2 * i + 2, :]
                                            .rearrange("p a b -> p (a b)"),
                                            ident[:])
                    nc.vector.scalar_tensor_tensor(out2[:, i, :],
                                                   psb.rearrange("p a b -> p (a b)"),
                                                   1.0, out1[:, i, :], OP.mult, OP.add)
                for bb in range(PB):
                    nc.sync.dma_start(y_d[b0 + bb],
                                      out2[(bb % 2) * T:(bb % 2) * T + T, bb // 2, :])

    nc.compile()
    return nc


def _prep_inputs(inputs):
    bf = ml_dtypes.bfloat16
    f64 = np.float64
    g1 = np.asarray(inputs["ln1_g"], f64)
    b1 = np.asarray(inputs["ln1_b"], f64)
    g2 = np.asarray(inputs["ln2_g"], f64)
    b2 = np.asarray(inputs["ln2_b"], f64)
    mk = np.asarray(inputs["att_mix_k"], f64).ravel()
    mv = np.asarray(inputs["att_mix_v"], f64).ravel()
    mr = np.asarray(inputs["att_mix_r"], f64).ravel()
    mkf = np.asarray(inputs["ffn_mix_k"], f64).ravel()
    mrf = np.asarray(inputs["ffn_mix_r"], f64).ravel()
    td = np.asarray(inputs["time_decay"], f64)
    u = np.asarray(inputs["time_first"], f64)
    Wk = np.asarray(inputs["Wk"], f64)
    Wv = np.asarray(inputs["Wv"], f64)
    Wr = np.asarray(inputs["Wr"], f64)
    Wo = np.asarray(inputs["Wo"], f64)
    Fk = np.asarray(inputs["Fk"], f64)
    Fv = np.asarray(inputs["Fv"], f64)
    Fr = np.asarray(inputs["Fr"], f64)

    def lhsT(W, colscale):
        return np.ascontiguousarray((W * colscale[None, :]).T.astype(np.float32)).astype(bf)

    d = {}
    d["wk_a"] = lhsT(Wk, g1 * (1 - mk))
    d["wk_b"] = lhsT(Wk, g1 * mk)
    d["wv_a"] = lhsT(Wv, g1 * (1 - mv))
    d["wv_b"] = lhsT(Wv, g1 * mv)
    d["wr_a"] = lhsT(Wr, g1 * (1 - mr))
    d["wr_b"] = lhsT(Wr, g1 * mr)
    d["wo_t"] = lhsT(Wo, np.ones(C))
    d["fr_a"] = lhsT(Fr, g2 * (1 - mrf))
    d["fr_b"] = lhsT(Fr, g2 * mrf)
    d["fk_t"] = lhsT(Fk, g2)
    d["fv_t"] = lhsT(0.5 * Fv, np.ones(H))

    def cols(vecs):
        # [C or H] vectors -> [128, nblk, nvec]
        n = vecs[0].shape[0]
        arr = np.stack(vecs, -1).reshape(n // 128, 128, len(vecs))
        return np.ascontiguousarray(arr.transpose(1, 0, 2)).astype(np.float32)

    ew = np.exp(-np.exp(td))
    eu = np.exp(u)
    d["colsA"] = cols([u, eu, ew, mkf, 1.0 - mkf])
    bk = Wk @ b1
    bkc = Wk @ (mk * b1)
    bv = Wv @ b1
    bvc = Wv @ (mv * b1)
    br = Wr @ b1
    brc = Wr @ (mr * b1)
    bfr = Fr @ b2
    bfrc = Fr @ (mrf * b2)
    d["colsD"] = cols([bk, bkc, bv, bvc, -br, -brc, 0.5 * bfr, 0.5 * bfrc])
    bfk = Fk @ b2
    bfkc = Fk @ (mkf * b2)
    d["colsH"] = cols([bfk, bfkc])
    return d


_NC_CACHE = [None]
_RUN_CACHE = [None]


def _make_runner():
    """Build the PJRT executable once (run_bass_via_pjrt re-traces per call)."""
    import jax
    import concourse.mybir as _mybir
    from concourse.bass2jax import install_neuronx_cc_hook, _bass_exec_p, partition_id_tensor
    from jax.sharding import Mesh, PartitionSpec
    from jax.experimental.shard_map import shard_map

    nc = _NC_CACHE[0]
    install_neuronx_cc_hook()
    partition_name = nc.partition_id_tensor.name if nc.partition_id_tensor else None
    in_names, out_names, out_avals = [], [], []
    for alloc in nc.m.functions[0].allocations:
        if not isinstance(alloc, _mybir.MemoryLocationSet):
            continue
        name = alloc.memorylocations[0].name
        if alloc.kind == "ExternalInput":
            if name != partition_name:
                in_names.append(name)
        elif alloc.kind == "ExternalOutput":
            out_names.append(name)
            out_avals.append(jax.core.ShapedArray(tuple(alloc.tensor_shape),
                                                  _mybir.dt.np(alloc.dtype)))
    n_params = len(in_names)
    all_names = list(in_names) + list(out_names)
    if partition_name is not None:
        all_names.append(partition_name)

    def _body(*args):
        operands = list(args)
        if partition_name is not None:
            operands.append(partition_id_tensor())
        return tuple(_bass_exec_p.bind(
            *operands, out_avals=tuple(out_avals), in_names=tuple(all_names),
            out_names=tuple(out_names), lowering_input_output_aliases=(),
            sim_require_finite=True, sim_require_nnan=True, nc=nc))

    devices = jax.devices()[:NCORE]
    mesh = Mesh(np.asarray(devices), ("core",))
    nio = n_params + len(out_names)
    sharded = jax.jit(
        shard_map(_body, mesh=mesh, in_specs=(PartitionSpec("core"),) * nio,
                  out_specs=(PartitionSpec("core"),) * len(out_names), check_rep=False),
        donate_argnums=tuple(range(n_params, nio)), keep_unused=True)
    return sharded, in_names, out_names, out_avals


def kernel(**inputs):
    import jax
    if _NC_CACHE[0] is None:
        _NC_CACHE[0] = _build()
        _RUN_CACHE[0] = _make_runner()
    sharded, in_names, out_names, out_avals = _RUN_CACHE[0]
    d = _prep_inputs(inputs)
    x = np.asarray(inputs["x"], np.float32)
    concat_in = []
    for name in in_names:
        if name == "x":
            concat_in.append(x.reshape(NCORE * BS, T, C))
        else:
            v = d[name]
            concat_in.append(np.broadcast_to(v, (NCORE,) + v.shape)
                             .reshape(NCORE * v.shape[0], *v.shape[1:]))
    zeros = [np.zeros((NCORE * a.shape[0], *a.shape[1:]), a.dtype) for a in out_avals]
    outs = sharded(*concat_in, *zeros)
    y = np.asarray(outs[out_names.index("y")])
    return y.reshape(NCORE, BS, T, C).reshape(B_FULL, T, C)


# revision 16
# speedup vs baseline: 1.2181x; 1.0501x over previous
"""RWKV v4 block kernel for 8 TRN2 NeuronCores (nn_Block_15083925144394).

Sharding: data-parallel over batch B=512 -> 64 per core, processed in 2
passes of 32 batch rows. Token-major LN on [100,512] tiles (2 batch rows),
channels-major matmuls/WKV with a 51-wide padded time axis so time-shifts
are plain AP offsets and the WKV recurrence runs as tensor_tensor_scan with
zero-multiplier state resets at batch boundaries.
"""
import os
import sys

sys.path.insert(0, "/opt/trn_rl_repo")

import numpy as np
import ml_dtypes

import concourse.bass as bass
import concourse.mybir as mybir
import concourse.tile as tile
from concourse import bacc
from concourse.bass_utils import run_bass_kernel_spmd
from concourse.masks import make_identity

F32 = mybir.dt.float32
BF16 = mybir.dt.bfloat16
AF = mybir.ActivationFunctionType
OP = mybir.AluOpType

NCORE = 8
B_FULL, T, C, H = 512, 50, 512, 2048
BS = B_FULL // NCORE          # 64 batch rows per core
PB = 16                       # batch rows per pass
NPASS = BS // PB              # 2
TP = T + 1                    # padded time width (col 0 is zero pad)
NT = PB // 2                  # 16 token tiles per pass (2 b-rows x 50 = 100 tokens each)
NTOK = 100                    # tokens per token-tile
CB = C // 128                 # 4 channel blocks
HB = H // 128                 # 16 hidden blocks
BCH = [(0, 10), (10, 16)]     # b-row chunks (<=500 tokens)

_EXEC_NS = [None]


class _OneSetBacc(bacc.Bacc):
    """Pin every activation to natural_log_exp_and_others (covers Copy,
    Identity, Exp, Ln, Relu, Square) so no ACT table reloads occur mid-kernel.
    Set ids are positional, so other sets are emptied rather than removed."""

    def insert_act_table_loads(self):
        import concourse.mybir as _mb
        from concourse.hw_specs import get_activation_tables
        from concourse import bacc as _bacc
        has_activation = any(
            isinstance(i, _mb.InstActivation)
            for b in self.main_func.blocks
            for i in b.instructions
        )
        if not has_activation:
            return
        tables = []
        for name, funcs in get_activation_tables(self.m.arch).items():
            tables.append((name, funcs if name == "natural_log_exp_and_others" else set()))
        _bacc._bass_rust.insert_act_table_loads(self, tables)


def _build():
    nc = _OneSetBacc("TRN2", target_bir_lowering=False, debug=False, num_devices=NCORE)

    x_d = nc.dram_tensor("x", [BS, T, C], F32, kind="ExternalInput")
    y_d = nc.dram_tensor("y", [BS, T, C], F32, kind="ExternalOutput")
    # weights, lhsT layout [c_in, c_out], bf16
    wd = {}
    for nm, shp in [("wk_a", [C, C]), ("wk_b", [C, C]), ("wv_a", [C, C]),
                    ("wv_b", [C, C]), ("wr_a", [C, C]), ("wr_b", [C, C]),
                    ("wo_t", [C, C]), ("fr_a", [C, C]), ("fr_b", [C, C]),
                    ("fk_t", [C, H]), ("fv_t", [H, C])]:
        wd[nm] = nc.dram_tensor(nm, shp, BF16, kind="ExternalInput")
    colsA_d = nc.dram_tensor("colsA", [128, CB, 5], F32, kind="ExternalInput")   # u, eu, ew, mkf, 1-mkf
    colsD_d = nc.dram_tensor("colsD", [128, CB, 8], F32, kind="ExternalInput")   # bk,bkc,bv,bvc,br2,brc2,bfr2,bfrc2
    colsH_d = nc.dram_tensor("colsH", [128, HB, 2], F32, kind="ExternalInput")   # bfk,bfkc

    with tile.TileContext(nc) as tc:
        with tc.tile_pool(name="wpool", bufs=1) as wp, \
             tc.tile_pool(name="big", bufs=1) as bigp, \
             tc.tile_pool(name="med", bufs=1) as medp, \
             tc.tile_pool(name="scr", bufs=2) as scrp, \
             tc.tile_pool(name="st", bufs=2) as stp, \
             tc.tile_pool(name="pmm", bufs=2, space="PSUM") as pmm, \
             tc.tile_pool(name="pkv", bufs=1, space="PSUM") as pkv, \
             tc.tile_pool(name="ptr", bufs=2, space="PSUM") as ptr:

            # ---- persistent constants ----
            ident = wp.tile([128, 128], BF16)
            make_identity(nc, ident[:])
            wt = {}
            for nm in ["wk_a", "wk_b", "wv_a", "wv_b", "wr_a", "wr_b", "wo_t", "fr_a", "fr_b"]:
                wt[nm] = wp.tile([128, CB, C], BF16, tag=nm, name=nm)
                nc.sync.dma_start(wt[nm][:], wd[nm].ap().rearrange("(a p) d -> p a d", p=128))
            wt["fk_t"] = wp.tile([128, CB, H], BF16, tag="fk_t", name="fk_t")
            nc.sync.dma_start(wt["fk_t"][:], wd["fk_t"].ap().rearrange("(a p) d -> p a d", p=128))
            wt["fv_t"] = wp.tile([128, HB, C], BF16, tag="fv_t", name="fv_t")
            nc.sync.dma_start(wt["fv_t"][:], wd["fv_t"].ap().rearrange("(a p) d -> p a d", p=128))
            epsc = wp.tile([128, 1], F32)
            nc.vector.memset(epsc[:], 1e-5)
            colsA = wp.tile([128, CB, 5], F32)
            colsD = wp.tile([128, CB, 8], F32)
            colsH = wp.tile([128, HB, 2], F32)
            nc.sync.dma_start(colsA[:], colsA_d.ap())
            nc.sync.dma_start(colsD[:], colsD_d.ap())
            nc.sync.dma_start(colsH[:], colsH_d.ap())
            u_c = lambda db: colsA[:, db, 0:1]
            eu_c = lambda db: colsA[:, db, 1:2]
            ew_c = lambda db: colsA[:, db, 2:3]

            # ONES feeds the per-db EW rebuild inside the WKV loop
            ONES = wp.tile([128, PB, T], BF16)
            nc.vector.memset(ONES[:], 1.0)

            for p in range(NPASS):
                b0 = p * PB
                # ================= Phase A: load + LN1 (token-major) =================
                x_tm = bigp.tile([NTOK, NT, C], F32, tag="xbig")
                for bb in range(PB):
                    nc.sync.dma_start(x_tm[(bb % 2) * T:(bb % 2) * T + T, bb // 2, :],
                                      x_d[b0 + bb])
                MV = stp.tile([NTOK, NT, 2], F32, tag="mv")
                for i in range(NT):
                    bst = stp.tile([NTOK, 6], F32, tag="bst")
                    nc.vector.bn_stats(bst[:], x_tm[:, i, :])
                    nc.vector.bn_aggr(MV[:, i, :], bst[:])
                LV = stp.tile([NTOK, NT], F32, tag="lv")
                nc.scalar.activation(LV[:], MV[:, :, 1:2], AF.Ln, bias=epsc[0:NTOK, :])
                RSTD = stp.tile([NTOK, NT], F32, tag="rstd")
                nc.scalar.activation(RSTD[:], LV[:], AF.Exp, bias=0.0, scale=-0.5)

                h1 = medp.tile([128, CB, PB, TP], BF16, tag="hcm")
                for cb in range(CB):
                    nc.vector.memset(h1[:, cb, :, 0:1], 0.0)
                for i in range(NT):
                    xh = scrp.tile([NTOK, C], F32, tag="xh")
                    nc.vector.tensor_scalar(xh[:], x_tm[:, i, :], MV[:, i, 0:1], None,
                                            OP.subtract)
                    xhb = scrp.tile([NTOK, C], BF16, tag="xhb")
                    nc.vector.tensor_scalar(xhb[:], xh[:], RSTD[:, i:i + 1], None, OP.mult)
                    pst = ptr.tile([128, CB, NTOK], BF16, tag="pst")
                    for cb in range(CB):
                        nc.tensor.transpose(pst[:, cb, :], xhb[:, cb * 128:(cb + 1) * 128],
                                            ident[0:NTOK, 0:NTOK])
                    for cb in range(CB):
                        nc.scalar.copy(h1[:, cb, 2 * i:2 * i + 2, 1:TP],
                                       pst[:, cb, :].rearrange("p (a b) -> p a b", a=2))


                # ============ Phase B: k/v/r matmuls + WKV, per output block ============
                rwkv = medp.tile([128, CB, PB, TP], BF16, tag="rwkv")
                for db in range(CB):
                    KD = medp.tile([128, PB, TP], F32, tag="kd", bufs=2)
                    VD = medp.tile([128, PB, TP], F32, tag="vd", bufs=2)
                    TH = medp.tile([128, PB, T], F32, tag="th")
                    for (wa, wb, dst, bcol, ext) in [
                            ("wk_a", "wk_b", KD, 0, True),
                            ("wv_a", "wv_b", VD, 2, True),
                            ("wr_a", "wr_b", TH, 4, False)]:
                        for (bl, bh) in BCH:
                            nb = bh - bl
                            ps = pmm.tile([128, 10, T], F32, tag="ps")
                            pso = ps[:, 0:nb, :].rearrange("p a b -> p (a b)")
                            for ci in range(CB):
                                nc.tensor.matmul(pso, wt[wa][:, ci, db * 128:(db + 1) * 128],
                                                 h1[:, ci, bl:bh, 0:T],
                                                 start=(ci == 0), stop=False)
                            for ci in range(CB):
                                nc.tensor.matmul(pso, wt[wb][:, ci, db * 128:(db + 1) * 128],
                                                 h1[:, ci, bl:bh, 1:TP],
                                                 start=False, stop=(ci == CB - 1))
                            if ext:  # k/v: affine evac with t=0 bias correction
                                nc.scalar.activation(dst[:, bl:bh, 2:TP], ps[:, 0:nb, 1:T],
                                                     AF.Identity, bias=colsD[:, db, bcol:bcol + 1])
                                nc.scalar.activation(dst[:, bl:bh, 1:2], ps[:, 0:nb, 0:1],
                                                     AF.Identity, bias=colsD[:, db, bcol + 1:bcol + 2])
                            else:  # r: E3 = exp(-(r + bias)) for sigmoid-fold
                                nc.scalar.activation(dst[:, bl:bh, 1:T], ps[:, 0:nb, 1:T],
                                                     AF.Exp, bias=colsD[:, db, 4:5], scale=-1.0)
                                nc.scalar.activation(dst[:, bl:bh, 0:1], ps[:, 0:nb, 0:1],
                                                     AF.Exp, bias=colsD[:, db, 5:6], scale=-1.0)
                    # WKV chain for this block
                    EK = medp.tile([128, PB, TP], F32, tag="ek", bufs=2)
                    nc.scalar.activation(EK[:, :, 1:TP], KD[:, :, 1:TP], AF.Exp)
                    EKV = medp.tile([128, PB, TP], F32, tag="ekv", bufs=2)
                    nc.vector.tensor_mul(EKV[:, :, 1:TP], EK[:, :, 1:TP], VD[:, :, 1:TP])
                    nc.vector.memset(EK[:, :, 0:1], 0.0)
                    nc.vector.memset(EKV[:, :, 0:1], 0.0)
                    EWd = medp.tile([128, PB, TP], F32, tag="ewd")
                    nc.vector.tensor_scalar(EWd[:, :, 1:TP], ONES[:], ew_c(db), None, OP.mult)
                    nc.vector.memset(EWd[:, :, 0:1], 0.0)
                    A = medp.tile([128, PB, TP], F32, tag="a")
                    nc.vector.tensor_tensor_scan(A.rearrange("p b t -> p (b t)"),
                                                 EWd.rearrange("p b t -> p (b t)"),
                                                 EKV.rearrange("p b t -> p (b t)"),
                                                 0.0, OP.mult, OP.add)
                    BB = medp.tile([128, PB, TP], F32, tag="bb")
                    nc.vector.tensor_tensor_scan(BB.rearrange("p b t -> p (b t)"),
                                                 EWd.rearrange("p b t -> p (b t)"),
                                                 EK.rearrange("p b t -> p (b t)"),
                                                 0.0, OP.mult, OP.add)
                    NUM = medp.tile([128, PB, T], F32, tag="num")
                    nc.vector.scalar_tensor_tensor(NUM[:], EKV[:, :, 1:TP], eu_c(db),
                                                   A[:, :, 0:T], OP.mult, OP.add)
                    DEN = medp.tile([128, PB, T], F32, tag="den")
                    nc.vector.scalar_tensor_tensor(DEN[:], EK[:, :, 1:TP], eu_c(db),
                                                   BB[:, :, 0:T], OP.mult, OP.add)
                    LD = medp.tile([128, PB, T], F32, tag="ld")
                    nc.scalar.activation(LD[:], DEN[:], AF.Ln)
                    L2 = medp.tile([128, PB, T], F32, tag="y")
                    nc.scalar.activation(L2[:], TH[:], AF.Ln, bias=1.0)
                    nc.vector.tensor_add(LD[:], LD[:], L2[:])
                    nc.scalar.activation(L2[:], LD[:], AF.Exp, bias=0.0, scale=-1.0)
                    nc.vector.tensor_mul(rwkv[:, db, :, 1:TP], NUM[:], L2[:])

                # ============ att = Wo @ rwkv, transpose back, residual ============
                attc = medp.tile([128, CB, PB, T], BF16, tag="dx")
                for db in range(CB):
                    for (bl, bh) in BCH:
                        nb = bh - bl
                        ps = pmm.tile([128, 10, T], F32, tag="ps")
                        pso = ps[:, 0:nb, :].rearrange("p a b -> p (a b)")
                        for ci in range(CB):
                            nc.tensor.matmul(pso, wt["wo_t"][:, ci, db * 128:(db + 1) * 128],
                                             rwkv[:, ci, bl:bh, 1:TP],
                                             start=(ci == 0), stop=(ci == CB - 1))
                        nc.scalar.copy(attc[:, db, bl:bh, :].rearrange("p a b -> p (a b)"),
                                       ps[:, 0:nb, :].rearrange("p a b -> p (a b)"))
                out1 = bigp.tile([NTOK, NT, C], F32, tag="out1")
                for i in range(NT):
                    psb = ptr.tile([NTOK, CB, 128], BF16, tag="pst")
                    for cb in range(CB):
                        nc.tensor.transpose(psb[:, cb, :],
                                            attc[:, cb, 2 * i:2 * i + 2, :]
                                            .rearrange("p a b -> p (a b)"),
                                            ident[:])
                    nc.vector.scalar_tensor_tensor(out1[:, i, :],
                                                   psb.rearrange("p a b -> p (a b)"),
                                                   1.0, x_tm[:, i, :], OP.mult, OP.add)

                # ================= Phase C: LN2 (token-major) =================
                MV2 = stp.tile([NTOK, NT, 2], F32, tag="mv")
                for i in range(NT):
                    bst = stp.tile([NTOK, 6], F32, tag="bst")
                    nc.vector.bn_stats(bst[:], out1[:, i, :])
                    nc.vector.bn_aggr(MV2[:, i, :], bst[:])
                LV2 = stp.tile([NTOK, NT], F32, tag="lv")
                nc.scalar.activation(LV2[:], MV2[:, :, 1:2], AF.Ln, bias=epsc[0:NTOK, :])
                RSTD2 = stp.tile([NTOK, NT], F32, tag="rstd")
                nc.scalar.activation(RSTD2[:], LV2[:], AF.Exp, bias=0.0, scale=-0.5)
                h2 = medp.tile([128, CB, PB, TP], BF16, tag="hcm2")
                for cb in range(CB):
                    nc.vector.memset(h2[:, cb, :, 0:1], 0.0)
                for i in range(NT):
                    xh = scrp.tile([NTOK, C], F32, tag="xh")
                    nc.vector.tensor_scalar(xh[:], out1[:, i, :], MV2[:, i, 0:1], None,
                                            OP.subtract)
                    xhb = scrp.tile([NTOK, C], BF16, tag="xhb")
                    nc.vector.tensor_scalar(xhb[:], xh[:], RSTD2[:, i:i + 1], None, OP.mult)
                    pst = ptr.tile([128, CB, NTOK], BF16, tag="pst")
                    for cb in range(CB):
                        nc.tensor.transpose(pst[:, cb, :], xhb[:, cb * 128:(cb + 1) * 128],
                                            ident[0:NTOK, 0:NTOK])
                    for cb in range(CB):
                        nc.scalar.copy(h2[:, cb, 2 * i:2 * i + 2, 1:TP],
                                       pst[:, cb, :].rearrange("p (a b) -> p a b", a=2))

                # ============ Phase D: FFN ============
                # fr path: frr = Fr@(h2sh + mrf*dx2) -> th2 = tanh(0.5 frr + 0.5 bias)
                th2 = medp.tile([128, CB, PB, T], BF16, tag="th2")
                for db in range(CB):
                    for (bl, bh) in BCH:
                        nb = bh - bl
                        ps = pmm.tile([128, 10, T], F32, tag="ps")
                        pso = ps[:, 0:nb, :].rearrange("p a b -> p (a b)")
                        for ci in range(CB):
                            nc.tensor.matmul(pso, wt["fr_a"][:, ci, db * 128:(db + 1) * 128],
                                             h2[:, ci, bl:bh, 0:T],
                                             start=(ci == 0), stop=False)
                        for ci in range(CB):
                            nc.tensor.matmul(pso, wt["fr_b"][:, ci, db * 128:(db + 1) * 128],
                                             h2[:, ci, bl:bh, 1:TP],
                                             start=False, stop=(ci == CB - 1))
                        nc.scalar.activation(th2[:, db, bl:bh, 1:T], ps[:, 0:nb, 1:T],
                                             AF.Exp, bias=colsD[:, db, 6:7], scale=-1.0)
                        nc.scalar.activation(th2[:, db, bl:bh, 0:1], ps[:, 0:nb, 0:1],
                                             AF.Exp, bias=colsD[:, db, 7:8], scale=-1.0)
                        nc.scalar.activation(th2[:, db, bl:bh, :], th2[:, db, bl:bh, :],
                                             AF.Ln, bias=1.0)
                        nc.scalar.activation(th2[:, db, bl:bh, :], th2[:, db, bl:bh, :],
                                             AF.Exp, bias=0.0, scale=-1.0)
                # fk / fv path with relu^2, streamed per h-block
                fkm = medp.tile([128, CB, PB, TP], BF16, tag="rwkv")
                for ci in range(CB):
                    fct = scrp.tile([128, PB, T], BF16, tag="fct")
                    nc.vector.tensor_scalar(fct[:], h2[:, ci, :, 1:TP], colsA[:, ci, 3:4],
                                            None, OP.mult)
                    nc.vector.scalar_tensor_tensor(fkm[:, ci, :, 1:TP], h2[:, ci, :, 0:T],
                                                   colsA[:, ci, 4:5], fct[:],
                                                   OP.mult, OP.add)
                rkv = medp.tile([128, CB, PB, T], BF16, tag="rkv")
                for (bl, bh) in BCH:
                    nb = bh - bl
                    pvs = [pkv.tile([128, 10, T], F32, tag=f"kv{cb}", name=f"kv{cb}") for cb in range(CB)]
                    for hb in range(HB):
                        ps = pmm.tile([128, 10, T], F32, tag="ps")
                        pso = ps[:, 0:nb, :].rearrange("p a b -> p (a b)")
                        for ci in range(CB):
                            nc.tensor.matmul(pso, wt["fk_t"][:, ci, hb * 128:(hb + 1) * 128],
                                             fkm[:, ci, bl:bh, 1:TP],
                                             start=(ci == 0), stop=(ci == CB - 1))
                        tkk = scrp.tile([128, 10, T], F32, tag="tkk")
                        nc.scalar.activation(tkk[:, 0:nb, 1:T], ps[:, 0:nb, 1:T],
                                             AF.Relu, bias=colsH[:, hb, 0:1])
                        nc.scalar.activation(tkk[:, 0:nb, 0:1], ps[:, 0:nb, 0:1],
                                             AF.Relu, bias=colsH[:, hb, 1:2])
                        kk = scrp.tile([128, 10, T], BF16, tag="kk")
                        nc.vector.tensor_mul(kk[:, 0:nb, :], tkk[:, 0:nb, :], tkk[:, 0:nb, :])
                        for cb in range(CB):
                            nc.tensor.matmul(pvs[cb][:, 0:nb, :].rearrange("p a b -> p (a b)"),
                                             wt["fv_t"][:, hb, cb * 128:(cb + 1) * 128],
                                             kk[:, 0:nb, :].rearrange("p a b -> p (a b)"),
                                             start=(hb == 0), stop=(hb == HB - 1))
                    for cb in range(CB):
                        nc.vector.tensor_mul(rkv[:, cb, bl:bh, :], th2[:, cb, bl:bh, :],
                                             pvs[cb][:, 0:nb, :])

                # ============ final: out2 = out1 + rkv^T, DMA out ============
                out2 = bigp.tile([NTOK, NT, C], F32, tag="xbig")
                for i in range(NT):
                    psb = ptr.tile([NTOK, CB, 128], BF16, tag="pst")
                    for cb in range(CB):
                        nc.tensor.transpose(psb[:, cb, :],
                                            rkv[:, cb, 2 * i:2 * i + 2, :]
                                            .rearrange("p a b -> p (a b)"),
                                            ident[:])
                    nc.vector.scalar_tensor_tensor(out2[:, i, :],
                                                   psb.rearrange("p a b -> p (a b)"),
                                                   1.0, out1[:, i, :], OP.mult, OP.add)
                for bb in range(PB):
                    nc.sync.dma_start(y_d[b0 + bb],
                                      out2[(bb % 2) * T:(bb % 2) * T + T, bb // 2, :])

    nc.compile()
    return nc


def _prep_inputs(inputs):
    bf = ml_dtypes.bfloat16
    f64 = np.float64
    g1 = np.asarray(inputs["ln1_g"], f64)
    b1 = np.asarray(inputs["ln1_b"], f64)
    g2 = np.asarray(inputs["ln2_g"], f64)
    b2 = np.asarray(inputs["ln2_b"], f64)
    mk = np.asarray(inputs["att_mix_k"], f64).ravel()
    mv = np.asarray(inputs["att_mix_v"], f64).ravel()
    mr = np.asarray(inputs["att_mix_r"], f64).ravel()
    mkf = np.asarray(inputs["ffn_mix_k"], f64).ravel()
    mrf = np.asarray(inputs["ffn_mix_r"], f64).ravel()
    td = np.asarray(inputs["time_decay"], f64)
    u = np.asarray(inputs["time_first"], f64)
    Wk = np.asarray(inputs["Wk"], f64)
    Wv = np.asarray(inputs["Wv"], f64)
    Wr = np.asarray(inputs["Wr"], f64)
    Wo = np.asarray(inputs["Wo"], f64)
    Fk = np.asarray(inputs["Fk"], f64)
    Fv = np.asarray(inputs["Fv"], f64)
    Fr = np.asarray(inputs["Fr"], f64)

    def lhsT(W, colscale):
        return np.ascontiguousarray((W * colscale[None, :]).T.astype(np.float32)).astype(bf)

    d = {}
    d["wk_a"] = lhsT(Wk, g1 * (1 - mk))
    d["wk_b"] = lhsT(Wk, g1 * mk)
    d["wv_a"] = lhsT(Wv, g1 * (1 - mv))
    d["wv_b"] = lhsT(Wv, g1 * mv)
    d["wr_a"] = lhsT(Wr, g1 * (1 - mr))
    d["wr_b"] = lhsT(Wr, g1 * mr)
    d["wo_t"] = lhsT(Wo, np.ones(C))
    d["fr_a"] = lhsT(Fr, g2 * (1 - mrf))
    d["fr_b"] = lhsT(Fr, g2 * mrf)
    d["fk_t"] = lhsT(Fk, g2)
    d["fv_t"] = lhsT(Fv, np.ones(H))

    def cols(vecs):
        # [C or H] vectors -> [128, nblk, nvec]
        n = vecs[0].shape[0]
        arr = np.stack(vecs, -1).reshape(n // 128, 128, len(vecs))
        return np.ascontiguousarray(arr.transpose(1, 0, 2)).astype(np.float32)

    ew = np.exp(-np.exp(td))
    eu = np.exp(u)
    d["colsA"] = cols([u, eu, ew, mkf, 1.0 - mkf])
    bk = Wk @ b1
    bkc = Wk @ (mk * b1)
    bv = Wv @ b1
    bvc = Wv @ (mv * b1)
    br = Wr @ b1
    brc = Wr @ (mr * b1)
    bfr = Fr @ b2
    bfrc = Fr @ (mrf * b2)
    d["colsD"] = cols([bk, bkc, bv, bvc, -br, -brc, -bfr, -bfrc])
    bfk = Fk @ b2
    bfkc = Fk @ (mkf * b2)
    d["colsH"] = cols([bfk, bfkc])
    return d


_NC_CACHE = [None]
_RUN_CACHE = [None]


def _make_runner():
    """Build the PJRT executable once (run_bass_via_pjrt re-traces per call)."""
    import jax
    import concourse.mybir as _mybir
    from concourse.bass2jax import install_neuronx_cc_hook, _bass_exec_p, partition_id_tensor
    from jax.sharding import Mesh, PartitionSpec
    from jax.experimental.shard_map import shard_map

    nc = _NC_CACHE[0]
    install_neuronx_cc_hook()
    partition_name = nc.partition_id_tensor.name if nc.partition_id_tensor else None
    in_names, out_names, out_avals = [], [], []
    for alloc in nc.m.functions[0].allocations:
        if not isinstance(alloc, _mybir.MemoryLocationSet):
            continue
        name = alloc.memorylocations[0].name
        if alloc.kind == "ExternalInput":
            if name != partition_name:
                in_names.append(name)
        elif alloc.kind == "ExternalOutput":
            out_names.append(name)
            out_avals.append(jax.core.ShapedArray(tuple(alloc.tensor_shape),
                                                  _mybir.dt.np(alloc.dtype)))
    n_params = len(in_names)
    all_names = list(in_names) + list(out_names)
    if partition_name is not None:
        all_names.append(partition_name)

    def _body(*args):
        operands = list(args)
        if partition_name is not None:
            operands.append(partition_id_tensor())
        return tuple(_bass_exec_p.bind(
            *operands, out_avals=tuple(out_avals), in_names=tuple(all_names),
            out_names=tuple(out_names), lowering_input_output_aliases=(),
            sim_require_finite=True, sim_require_nnan=True, nc=nc))

    devices = jax.devices()[:NCORE]
    mesh = Mesh(np.asarray(devices), ("core",))
    nio = n_params + len(out_names)
    sharded = jax.jit(
        shard_map(_body, mesh=mesh, in_specs=(PartitionSpec("core"),) * nio,
                  out_specs=(PartitionSpec("core"),) * len(out_names), check_rep=False),
        donate_argnums=tuple(range(n_params, nio)), keep_unused=True)
    return sharded, in_names, out_names, out_avals


def kernel(**inputs):
    import jax
    if _NC_CACHE[0] is None:
        _NC_CACHE[0] = _build()
        _RUN_CACHE[0] = _make_runner()
    sharded, in_names, out_names, out_avals = _RUN_CACHE[0]
    d = _prep_inputs(inputs)
    x = np.asarray(inputs["x"], np.float32)
    concat_in = []
    for name in in_names:
        if name == "x":
            concat_in.append(x.reshape(NCORE * BS, T, C))
        else:
            v = d[name]
            concat_in.append(np.broadcast_to(v, (NCORE,) + v.shape)
                             .reshape(NCORE * v.shape[0], *v.shape[1:]))
    zeros = [np.zeros((NCORE * a.shape[0], *a.shape[1:]), a.dtype) for a in out_avals]
    outs = sharded(*concat_in, *zeros)
    y = np.asarray(outs[out_names.index("y")])
    return y.reshape(NCORE, BS, T, C).reshape(B_FULL, T, C)
